# revision 1
# baseline (speedup 1.0000x reference)
"""CRF (Viterbi decode + log-likelihood) Bass/Tile kernel for Trainium2.

Problem: B=256, T=512, K=128 linear-chain CRF.
  loss   = sum_b -(sequence_score(b) - log_norm(b))
  preds  = viterbi decode tags [B, T] (int32)

Sharding: pure data parallel — batch split 32/core across 8 NeuronCores;
the [K,K] transition matrix and small index/mask constants are replicated.

Per-core on-chip layouts:
  A-space:   [32_b partitions, 128_k free]      (lognorm, scores, emissions)
  C-space:   [(4b+jq)_p, jr], j = 32*jq + jr    (viterbi alpha / backptrs)
  rep-space: [(4b+jq)_p, 128_i]                 (alpha replicated 4x per b)

The Viterbi max-plus inner product runs on the vector engine with two custom
DVE ops per 32-column segment: a fused add+max (accum) and a fused
first-index argmax (select(eq(Src0+Src1, max), Idx, BIG) with MIN-accum) —
bit-exact w.r.t. the float32 reference including tie-breaks. The logsumexp
recursion runs as exp (ACT) -> transpose+matmul (PE) -> log (ACT) with a
running per-example shift. Sequence scores use one-hot compare + fused
multiply-accumulate gathers. Backtrace uses one-hot gathers plus tiny PE
reductions.
"""
import json
import numpy as np

import concourse.bass as bass
import concourse.mybir as mybir
import concourse.tile as tile
import concourse.bass_utils as bass_utils
import concourse.bass2jax as bass2jax
from concourse.bass_utils import run_bass_kernel_spmd
from concourse.masks import make_identity

import concourse.dve_ops as dve_ops_mod
from concourse.dve_ops import DveOp, OPS
from concourse.dve_spec import (
    Spec, Src0, Src1, C0, C1, Idx, AluOp, eq, select, lower,
)
from concourse.dve_uop import DveOpSpec

F32 = mybir.dt.float32
I32 = mybir.dt.int32
AF = mybir.ActivationFunctionType
ALU = mybir.AluOpType
AX = mybir.AxisListType

B, T, K = 256, 512, 128
NCORES = 8
BC = B // NCORES
BIG = 3.4e38
NEG = -3.4e38
M_REFRESH = 4

# --------------------------------------------------------------------------
# compile workaround 1: this container's walrus accepts only ONE sync-wait
# per instruction; move extra waits onto inserted same-engine Drains.
# --------------------------------------------------------------------------


def _split_multiwaits_json(bir_json):
    d = json.loads(bir_json)
    for fn in d.get("functions", []):
        for bb in fn.get("blocks", []):
            out = []
            for ins in bb.get("instructions", []):
                si = ins.get("sync_info")
                waits = (si or {}).get("on_wait") or []
                if len(waits) > 1:
                    for k, w in enumerate(waits[:-1]):
                        out.append({
                            "debug": ins.get("debug", 0),
                            "engine": ins["engine"],
                            "ins": [], "outs": [],
                            "name": f"{ins['name']}-wsp{k}",
                            "opcode": "Drain",
                            "sync_info": {"on_update": [], "on_wait": [w]},
                        })
                    si["on_wait"] = [waits[-1]]
                out.append(ins)
            bb["instructions"] = out
    return json.dumps(d).encode()


_orig_compile_bir_kernel = bass_utils.compile_bir_kernel


def _patched_compile_bir_kernel(bir_json, tmpdir, neff_name="file.neff"):
    return _orig_compile_bir_kernel(_split_multiwaits_json(bir_json), tmpdir,
                                    neff_name)


def _apply_compile_patch():
    if bass_utils.compile_bir_kernel is not _patched_compile_bir_kernel:
        bass_utils.compile_bir_kernel = _patched_compile_bir_kernel
        bass2jax.compile_bir_kernel = _patched_compile_bir_kernel


_apply_compile_patch()

# --------------------------------------------------------------------------
# custom DVE ops (self-pinned shas)
# --------------------------------------------------------------------------


def _self_pinned(name, spec):
    shas = {}
    for ver in ("v3", "v4"):
        try:
            s = DveOpSpec(name=name, opcode=1, uops=lower(spec, ver=ver),
                          rd1_en=True)
            shas[ver] = s.sha(ver)
        except Exception:
            pass
    return DveOp(name, spec, subdim=False, uops_sha=shas)


def _ref_max2(in0, in1, c0, c1, c2):
    s = (in0 + in1).astype(np.float32)
    return s, np.maximum(np.max(s, axis=-1, keepdims=True), c0)


def _ref_argmax2(in0, in1, c0, c1, c2):
    idx = np.arange(in0.shape[-1], dtype=np.float32)
    w = np.where((in0 + in1).astype(np.float32) == c0, idx, c1)
    return w, np.minimum(np.minimum.reduce(w, axis=-1, keepdims=True), c1)


def _ref_argmax1(in0, in1, c0, c1, c2):
    idx = np.arange(in0.shape[-1], dtype=np.float32)
    w = np.where(in0 == c0, idx, c1)
    return w, np.minimum(np.minimum.reduce(w, axis=-1, keepdims=True), c1)


_REGISTERED = {}


def _register_ops():
    if not _REGISTERED:
        _REGISTERED["VIT_MAX"] = _self_pinned(
            "VIT_MAX", Spec(body=Src0 + Src1, accum=AluOp.MAX, accum_init=C0,
                            reference=_ref_max2))
        _REGISTERED["VIT_ARGMAX"] = _self_pinned(
            "VIT_ARGMAX", Spec(body=select(eq(Src0 + Src1, C0), Idx, C1),
                               accum=AluOp.MIN, accum_init=C1,
                               reference=_ref_argmax2))
        _REGISTERED["ARGMAX1"] = _self_pinned(
            "ARGMAX1", Spec(body=select(eq(Src0, C0), Idx, C1),
                            accum=AluOp.MIN, accum_init=C1,
                            reference=_ref_argmax1))
        for op in _REGISTERED.values():
            if op.name not in dve_ops_mod._SUB_OPCODE_FOR_NAME:
                OPS.append(op)
                dve_ops_mod._SUB_OPCODE_FOR_NAME[op.name] = (
                    dve_ops_mod._CUSTOM_DVE_ROW_BASE + len(OPS) - 1)
                dve_ops_mod.CUSTOM_DVE_SPECS[op.name] = op.spec
    return (_REGISTERED["VIT_MAX"], _REGISTERED["VIT_ARGMAX"],
            _REGISTERED["ARGMAX1"])


# --------------------------------------------------------------------------
# kernel build
# --------------------------------------------------------------------------


def build_nc(T=T):
    VIT_MAX, VIT_ARGMAX, ARGMAX1 = _register_ops()
    from contextlib import ExitStack

    nc = bass.Bass()
    logits_d = nc.dram_tensor("logits", [BC, T, K], F32, kind="ExternalInput")
    trans_d = nc.dram_tensor("trans", [K, K], F32, kind="ExternalInput")
    ttrep_d = nc.dram_tensor("ttrep", [K, 32 * K], F32, kind="ExternalInput")
    labf_d = nc.dram_tensor("labf", [BC, T], F32, kind="ExternalInput")
    mask_d = nc.dram_tensor("maskA", [BC, T], F32, kind="ExternalInput")
    imask_d = nc.dram_tensor("imaskA", [BC, T], F32, kind="ExternalInput")
    maskr_d = nc.dram_tensor("maskR", [K, T], F32, kind="ExternalInput")
    imaskr_d = nc.dram_tensor("imaskR", [K, T], F32, kind="ExternalInput")
    R_d = nc.dram_tensor("Rmat", [BC, K], F32, kind="ExternalInput")
    L_d = nc.dram_tensor("Lmat", [K, 4 * K], F32, kind="ExternalInput")
    GR_d = nc.dram_tensor("GRmat", [K, K], F32, kind="ExternalInput")
    G4_d = nc.dram_tensor("G4mat", [K, BC], F32, kind="ExternalInput")
    iotaA_d = nc.dram_tensor("iotaA", [BC, K], F32, kind="ExternalInput")
    iotaC_d = nc.dram_tensor("iotaC", [K, 32], F32, kind="ExternalInput")
    ll_d = nc.dram_tensor("ll", [BC, 1], F32, kind="ExternalOutput")
    pred_d = nc.dram_tensor("pred", [BC, T], I32, kind="ExternalOutput")

    with tile.TileContext(nc) as tc, ExitStack() as ctx:
        const = ctx.enter_context(tc.tile_pool(name="const", bufs=1))
        big = ctx.enter_context(tc.tile_pool(name="big", bufs=1))
        emitp = ctx.enter_context(tc.tile_pool(name="emit", bufs=3))
        small = ctx.enter_context(tc.tile_pool(name="small", bufs=3))
        arep_p = ctx.enter_context(tc.tile_pool(name="arep", bufs=2))
        alc_p = ctx.enter_context(tc.tile_pool(name="alc", bufs=2))
        lgn_p = ctx.enter_context(tc.tile_pool(name="lgn", bufs=3))
        bt_p = ctx.enter_context(tc.tile_pool(name="bt", bufs=2))
        ps1 = ctx.enter_context(tc.tile_pool(name="ps1", bufs=2, space="PSUM"))
        ps2 = ctx.enter_context(tc.tile_pool(name="ps2", bufs=2, space="PSUM"))
        ps3 = ctx.enter_context(tc.tile_pool(name="ps3", bufs=2, space="PSUM"))

        dma = nc.gpsimd.dma_start

        ttrep = const.tile([K, 32, K], F32)
        dma(ttrep[:], ttrep_d[:].rearrange("k (a b) -> k a b", a=32))
        trans = const.tile([K, K], F32); dma(trans[:], trans_d[:])
        labf = const.tile([BC, T], F32); dma(labf[:], labf_d[:])
        maskA = const.tile([BC, T], F32); dma(maskA[:], mask_d[:])
        imaskA = const.tile([BC, T], F32); dma(imaskA[:], imask_d[:])
        maskR = const.tile([K, T], F32); dma(maskR[:], maskr_d[:])
        imaskR = const.tile([K, T], F32); dma(imaskR[:], imaskr_d[:])
        Rm = const.tile([BC, K], F32); dma(Rm[:], R_d[:])
        Lm = const.tile([K, 4, K], F32)
        dma(Lm[:], L_d[:].rearrange("k (a b) -> k a b", a=4))
        GRm = const.tile([K, K], F32); dma(GRm[:], GR_d[:])
        G4m = const.tile([K, BC], F32); dma(G4m[:], G4_d[:])
        iotaA = const.tile([BC, K], F32); dma(iotaA[:], iotaA_d[:])
        iotaC = const.tile([K, 32], F32); dma(iotaC[:], iotaC_d[:])
        ident32 = const.tile([32, 32], F32)
        make_identity(nc, ident32[:])

        Emat = const.tile([K, K], F32)
        nc.scalar.activation(Emat[:], trans[:], AF.Exp, bias=0.0, scale=1.0)

        bpC = big.tile([K, T, 32], F32)
        uacc = const.tile([BC, T], F32)
        bacc = const.tile([BC, T], F32)
        predf = const.tile([BC, T], F32)
        dumpK = const.tile([K, 1], F32)
        sjunkA = const.tile([BC, K], F32)
        btjunk = const.tile([K, 32], F32)
        Mcol = const.tile([BC, 1], F32)
        outA = const.tile([BC, K], F32)
        outM = const.tile([BC, 1], F32)

        T_CH = 32
        echunks = {}

        def emit_t(t):
            c = t // T_CH
            if c not in echunks:
                tl = emitp.tile([BC, T_CH * K], F32, tag="echunk")
                t0 = c * T_CH
                t1 = min(T, t0 + T_CH)
                dma(tl[:, : (t1 - t0) * K],
                    logits_d[:, t0:t1, :].rearrange("b t k -> b (t k)"))
                echunks[c] = tl
            i = t % T_CH
            return echunks[c][:, i * K:(i + 1) * K]

        # ---- t = 0 init ----
        e0 = emit_t(0)
        ps0 = ps1.tile([K, K], F32, tag="parep")
        nc.tensor.matmul(ps0[:], Rm[:], e0, start=True, stop=True)
        arep = arep_p.tile([K, K], F32, tag="arep")
        nc.vector.tensor_copy(arep[:], ps0[:])

        A = lgn_p.tile([BC, K], F32, tag="A")
        nc.vector.tensor_copy(A[:], e0)
        nc.vector.memset(Mcol[:], 0.0)
        nc.vector.tensor_copy(outA[:], e0)
        nc.vector.memset(outM[:], 0.0)

        nc.vector.scalar_tensor_tensor(
            out=sjunkA[:], in0=iotaA[:], scalar=labf[:, 0:1], in1=e0,
            op0=ALU.is_equal, op1=ALU.mult, accum_out=uacc[:, 0:1])
        onehot = small.tile([BC, K], F32, tag="onehot")
        nc.vector.tensor_scalar(out=onehot[:], in0=iotaA[:],
                                scalar1=labf[:, 0:1], scalar2=None,
                                op0=ALU.is_equal)
        nc.vector.memset(bacc[:, 0:1], 0.0)

        # ---- main time loop ----
        for t in range(1, T):
            et = emit_t(t)

            alphaC = alc_p.tile([K, 32], F32, tag="alphaC")
            for jr in range(32):
                nc.vector._custom_dve(
                    VIT_MAX, out=dumpK[:].broadcast_to([K, K]),
                    in0=ttrep[:, jr, :], in1=arep[:], s0=NEG,
                    accum_out=alphaC[:, jr:jr+1])
            for jr in range(32):
                nc.vector._custom_dve(
                    VIT_ARGMAX, out=dumpK[:].broadcast_to([K, K]),
                    in0=ttrep[:, jr, :], in1=arep[:],
                    s0=alphaC[:, jr:jr+1], s1=BIG,
                    accum_out=bpC[:, t - 1, jr:jr+1])

            psr = ps1.tile([K, K], F32, tag="parep")
            for jq in range(4):
                nc.tensor.matmul(psr[:, 32*jq:32*(jq+1)], Lm[:, jq, :],
                                 alphaC[:], start=True, stop=True)
            psE = ps1.tile([K, K], F32, tag="pse")
            nc.tensor.matmul(psE[:], Rm[:], et, start=True, stop=True)
            e_s = small.tile([K, K], F32, tag="e_s")
            nc.vector.tensor_copy(e_s[:], psE[:])
            s1 = small.tile([K, K], F32, tag="s1")
            nc.vector.tensor_add(s1[:], psr[:], e_s[:])
            frozen = small.tile([K, K], F32, tag="frozen")
            nc.vector.tensor_scalar_mul(frozen[:], arep[:], imaskR[:, t:t+1])
            arep_n = arep_p.tile([K, K], F32, tag="arep")
            nc.vector.scalar_tensor_tensor(
                out=arep_n[:], in0=s1[:], scalar=maskR[:, t:t+1],
                in1=frozen[:], op0=ALU.mult, op1=ALU.add)
            arep = arep_n

            eA = lgn_p.tile([BC, K], F32, tag="eA")
            nc.scalar.activation(eA[:], A[:], AF.Exp, bias=0.0, scale=1.0)
            psT = ps2.tile([K, BC], F32, tag="tp32")
            nc.tensor.transpose(psT[:], eA[:], ident32[:])
            eT = lgn_p.tile([K, BC], F32, tag="eTs")
            nc.scalar.copy(eT[:], psT[:])
            psS = ps3.tile([BC, K], F32, tag="bck")
            nc.tensor.matmul(psS[:], eT[:], Emat[:], start=True, stop=True)
            lnS = lgn_p.tile([BC, K], F32, tag="lnS")
            nc.scalar.activation(lnS[:], psS[:], AF.Ln, bias=0.0, scale=1.0)
            A_n = lgn_p.tile([BC, K], F32, tag="A")
            nc.vector.tensor_add(A_n[:], lnS[:], et)
            ofrz = lgn_p.tile([BC, K], F32, tag="ofrz")
            nc.vector.tensor_scalar_mul(ofrz[:], outA[:], imaskA[:, t:t+1])
            nc.vector.scalar_tensor_tensor(
                out=outA[:], in0=A_n[:], scalar=maskA[:, t:t+1], in1=ofrz[:],
                op0=ALU.mult, op1=ALU.add)
            mfrz = lgn_p.tile([BC, 1], F32, tag="mfrz")
            nc.vector.tensor_scalar_mul(mfrz[:], outM[:], imaskA[:, t:t+1])
            nc.vector.scalar_tensor_tensor(
                out=outM[:], in0=Mcol[:], scalar=maskA[:, t:t+1], in1=mfrz[:],
                op0=ALU.mult, op1=ALU.add)
            A = A_n
            if t % M_REFRESH == 0:
                negm = lgn_p.tile([BC, 1], F32, tag="negm")
                nc.vector.tensor_reduce(negm[:], A[:], axis=AX.X, op=ALU.max,
                                        negate=True)
                A_s = lgn_p.tile([BC, K], F32, tag="A")
                nc.vector.tensor_scalar_add(A_s[:], A[:], negm[:])
                nc.vector.tensor_scalar_sub(Mcol[:], Mcol[:], negm[:])
                A = A_s

            nc.vector.scalar_tensor_tensor(
                out=sjunkA[:], in0=iotaA[:], scalar=labf[:, t:t+1], in1=et,
                op0=ALU.is_equal, op1=ALU.mult, accum_out=uacc[:, t:t+1])
            psO = ps2.tile([K, BC], F32, tag="tp32")
            nc.tensor.transpose(psO[:], onehot[:], ident32[:])
            ohT = small.tile([K, BC], F32, tag="ohTs")
            nc.scalar.copy(ohT[:], psO[:])
            psRow = ps3.tile([BC, K], F32, tag="bck")
            nc.tensor.matmul(psRow[:], ohT[:], trans[:], start=True, stop=True)
            onehot_n = small.tile([BC, K], F32, tag="onehot")
            nc.vector.tensor_scalar(out=onehot_n[:], in0=iotaA[:],
                                    scalar1=labf[:, t:t+1], scalar2=None,
                                    op0=ALU.is_equal)
            nc.vector.scalar_tensor_tensor(
                out=sjunkA[:], in0=onehot_n[:], scalar=1.0, in1=psRow[:],
                op0=ALU.mult, op1=ALU.mult, accum_out=bacc[:, t:t+1])
            onehot = onehot_n

        # ---- final tag + backtrace ----
        mfin = small.tile([K, 1], F32, tag="mfin")
        nc.vector.tensor_reduce(mfin[:], arep[:], axis=AX.X, op=ALU.max)
        tagrep = bt_p.tile([K, 1], F32, tag="tagrep")
        nc.vector._custom_dve(ARGMAX1, out=dumpK[:].broadcast_to([K, K]),
                              in0=arep[:], s0=mfin[:], s1=BIG,
                              accum_out=tagrep[:])
        psP = ps3.tile([BC, 1], F32, tag="bck")
        nc.tensor.matmul(psP[:], G4m[:], tagrep[:], start=True, stop=True)
        nc.vector.tensor_copy(predf[:, T-1:T], psP[:])
        onehotC = bt_p.tile([K, 32], F32, tag="onehotC")
        nc.vector.tensor_scalar(out=onehotC[:], in0=iotaC[:],
                                scalar1=tagrep[:], scalar2=None,
                                op0=ALU.is_equal)

        for t in range(T - 1, 0, -1):
            partial = bt_p.tile([K, 1], F32, tag="partial")
            nc.vector.scalar_tensor_tensor(
                out=btjunk[:], in0=bpC[:, t - 1, :], scalar=1.0,
                in1=onehotC[:], op0=ALU.mult, op1=ALU.mult,
                accum_out=partial[:])
            psG = ps2.tile([K, 1], F32, tag="tp32")
            nc.tensor.matmul(psG[:], GRm[:], partial[:], start=True, stop=True)
            tfrz = bt_p.tile([K, 1], F32, tag="tfrz")
            nc.vector.tensor_scalar_mul(tfrz[:], tagrep[:], imaskR[:, t:t+1])
            tag_n = bt_p.tile([K, 1], F32, tag="tagrep")
            nc.vector.scalar_tensor_tensor(
                out=tag_n[:], in0=psG[:], scalar=maskR[:, t:t+1], in1=tfrz[:],
                op0=ALU.mult, op1=ALU.add)
            tagrep = tag_n
            onehotC_n = bt_p.tile([K, 32], F32, tag="onehotC")
            nc.vector.tensor_scalar(out=onehotC_n[:], in0=iotaC[:],
                                    scalar1=tagrep[:], scalar2=None,
                                    op0=ALU.is_equal)
            onehotC = onehotC_n
            psP2 = ps3.tile([BC, 1], F32, tag="bck")
            nc.tensor.matmul(psP2[:], G4m[:], tagrep[:], start=True, stop=True)
            nc.vector.tensor_copy(predf[:, t-1:t], psP2[:])

        # ---- finalize ----
        negmF = small.tile([BC, 1], F32, tag="negmF")
        nc.vector.tensor_reduce(negmF[:], outA[:], axis=AX.X, op=ALU.max,
                                negate=True)
        sF = small.tile([BC, 1], F32, tag="sF")
        eF = small.tile([BC, K], F32, tag="eF")
        nc.scalar.activation(eF[:], outA[:], AF.Exp, bias=negmF[:], scale=1.0,
                             accum_out=sF[:])
        lnF = small.tile([BC, 1], F32, tag="lnF")
        nc.scalar.activation(lnF[:], sF[:], AF.Ln, bias=0.0, scale=1.0)
        logZ = small.tile([BC, 1], F32, tag="logZ")
        nc.vector.tensor_sub(logZ[:], lnF[:], negmF[:])
        nc.vector.tensor_add(logZ[:], logZ[:], outM[:])
        um = small.tile([BC, T], F32, tag="um")
        nc.vector.tensor_mul(um[:], uacc[:], maskA[:])
        usum = small.tile([BC, 1], F32, tag="usum")
        nc.vector.tensor_reduce(usum[:], um[:], axis=AX.X, op=ALU.add)
        bm = small.tile([BC, T], F32, tag="bm")
        nc.vector.tensor_mul(bm[:], bacc[:], maskA[:])
        bsum = small.tile([BC, 1], F32, tag="bsum")
        nc.vector.tensor_reduce(bsum[:], bm[:], axis=AX.X, op=ALU.add)
        ll = small.tile([BC, 1], F32, tag="ll")
        nc.vector.tensor_add(ll[:], usum[:], bsum[:])
        nc.vector.tensor_sub(ll[:], ll[:], logZ[:])
        dma(ll_d[:], ll[:])
        predi = const.tile([BC, T], I32)
        nc.vector.tensor_copy(predi[:], predf[:])
        dma(pred_d[:], predi[:])

    mybir.codegen_inst_isa_subclasses(nc)
    return nc


# --------------------------------------------------------------------------
# host glue
# --------------------------------------------------------------------------


def make_host_inputs(logits, labels, seq_lens, trans_params, T=T):
    logits = np.ascontiguousarray(logits, dtype=np.float32)
    labels = np.asarray(labels)
    seq_lens = np.asarray(seq_lens).astype(np.int32)
    trans = np.ascontiguousarray(trans_params, dtype=np.float32)

    tt = trans.T
    ttrep = np.empty((K, 32 * K), dtype=np.float32)
    for p in range(K):
        jq = p % 4
        ttrep[p] = tt[32 * jq: 32 * jq + 32, :].reshape(-1)
    Rmat = np.zeros((BC, K), dtype=np.float32)
    for b in range(BC):
        Rmat[b, 4 * b:4 * b + 4] = 1.0
    Lmat = np.zeros((4, K, K), dtype=np.float32)
    for jq in range(4):
        for p in range(K):
            Lmat[jq, 4 * (p // 4) + jq, p] = 1.0
    Lhost = np.transpose(Lmat, (1, 0, 2)).reshape(K, 4 * K).copy()
    GRmat = np.zeros((K, K), dtype=np.float32)
    for c in range(K):
        GRmat[c, (c // 4) * 4:(c // 4) * 4 + 4] = 1.0
    G4mat = np.zeros((K, BC), dtype=np.float32)
    for c in range(K):
        G4mat[c, c // 4] = 0.25
    iotaA = np.tile(np.arange(K, dtype=np.float32), (BC, 1))
    iotaC = np.empty((K, 32), dtype=np.float32)
    for p in range(K):
        iotaC[p] = 32 * (p % 4) + np.arange(32, dtype=np.float32)

    in_maps = []
    for c in range(NCORES):
        sl = slice(c * BC, (c + 1) * BC)
        lab = labels[sl].astype(np.float32)
        lens = seq_lens[sl]
        maskA = (np.arange(T)[None, :] < lens[:, None]).astype(np.float32)
        maskR = np.repeat(maskA, 4, axis=0)
        in_maps.append(dict(
            logits=logits[sl], trans=trans, ttrep=ttrep,
            labf=lab, maskA=maskA, imaskA=(1.0 - maskA).astype(np.float32),
            maskR=maskR, imaskR=(1.0 - maskR).astype(np.float32),
            Rmat=Rmat, Lmat=Lhost, GRmat=GRmat, G4mat=G4mat,
            iotaA=iotaA, iotaC=iotaC,
        ))
    return in_maps


_NC_CACHE = {}


def _get_nc(T=T):
    if T not in _NC_CACHE:
        _NC_CACHE[T] = build_nc(T)
    return _NC_CACHE[T]


def run(logits, labels, seq_lens, trans_params, T=T, **spmd_kwargs):
    nc = _get_nc(T)
    in_maps = make_host_inputs(logits, labels, seq_lens, trans_params, T)
    res = run_bass_kernel_spmd(nc, in_maps, core_ids=list(range(NCORES)),
                               **spmd_kwargs)
    ll = np.concatenate([r["ll"][:, 0] for r in res.results])
    pred = np.concatenate([r["pred"] for r in res.results], axis=0)
    loss = np.float32(-(ll.astype(np.float64).sum()))
    return loss, pred.astype(np.int32), res


def kernel(logits, labels, seq_lens, trans_params):
    loss, pred, _ = run(np.asarray(logits), np.asarray(labels),
                        np.asarray(seq_lens), np.asarray(trans_params))
    return loss, pred
